# revision 1
# baseline (speedup 1.0000x reference)
"""Trainium2 Bass kernel for nn_Actor (diagonal complex LRU, last-step output).

Math: the reference runs an associative scan x_t = lam*x_{t-1} + (gamma*B) u_t
over L=2048 steps and keeps only y[:, -1, :].  The last state collapses to
    x_L[n] = sum_t lam[n]^(L-1-t) * (Bmat @ u_t)[n]
which we reorder as
    v[n, b, h] = sum_t W[t, n] * u[b, t, h]      (TensorE, contracts time)
    x[n, b]    = sum_h Bmat[n, h] * v[n, b, h]   (VectorE, fused mul+reduce)
    y[b, o]    = Re(C x)[b, o] + (D u_last)[b, o] (TensorE, tiny)
with W[t, n] = lam[n]^(L-1-t) generated on-device from nu/theta logs.

Sharding: data-parallel over batch (64 -> 8 per core) on 8 NeuronCores,
no collectives; host concatenates per-core outputs.
"""

import sys

sys.path.insert(0, "/opt/trn_rl_repo")

import math

import numpy as np

import concourse.bass as bass
import concourse.tile as tile
from concourse import bacc, mybir
from concourse.bass_utils import run_bass_kernel_spmd

B, L, H, O, N = 64, 2048, 128, 128, 256
NCORES = 8
BS = B // NCORES  # 8 batches per core
KT = L // 128  # 16 time tiles of 128
F32 = mybir.dt.float32
F32R = mybir.dt.float32r
I32 = mybir.dt.int32
BF16 = mybir.dt.bfloat16
MULT = mybir.AluOpType.add  # placeholder; real ops below
PI = math.pi


def build(stage=5):
    nc = bacc.Bacc("TRN2", target_bir_lowering=False, debug=False)

    u_d = nc.dram_tensor("u", [BS, L, H], F32, kind="ExternalInput")
    nu_d = nc.dram_tensor("nu_log", [N], F32, kind="ExternalInput")
    th_d = nc.dram_tensor("theta_log", [N], F32, kind="ExternalInput")
    gm_d = nc.dram_tensor("gamma_log", [N], F32, kind="ExternalInput")
    bre_d = nc.dram_tensor("B_re", [N, H], F32, kind="ExternalInput")
    bim_d = nc.dram_tensor("B_im", [N, H], F32, kind="ExternalInput")
    cre_d = nc.dram_tensor("C_re", [O, N], F32, kind="ExternalInput")
    cim_d = nc.dram_tensor("C_im", [O, N], F32, kind="ExternalInput")
    dd_d = nc.dram_tensor("D", [O, H], F32, kind="ExternalInput")
    iota_d = nc.dram_tensor("iota128", [128, 1], F32, kind="ExternalInput")
    ident_d = nc.dram_tensor("ident128", [128, 128], F32, kind="ExternalInput")
    out_d = nc.dram_tensor("out", [BS, O], F32, kind="ExternalOutput")

    mult = mybir.AluOpType.mult
    add = mybir.AluOpType.add
    sub = mybir.AluOpType.subtract
    Act = mybir.ActivationFunctionType

    with tile.TileContext(nc) as tc:
        with (
            tc.tile_pool(name="const", bufs=1) as cp,
            tc.tile_pool(name="upool", bufs=8) as up,
            tc.tile_pool(name="wk", bufs=1) as wk,
            tc.tile_pool(name="psum", bufs=1, space=bass.MemorySpace.PSUM) as pp,
        ):
            # ---- params -> rows, f32 broadcasts for seed generation --------
            nu_row = cp.tile([1, N], F32, tag="nu_row")
            th_row = cp.tile([1, N], F32, tag="th_row")
            nc.sync.dma_start(nu_row[:], nu_d[None, :])
            nc.sync.dma_start(th_row[:], th_d[None, :])
            iota_f = cp.tile([128, 1], F32, tag="iota_f")
            nc.sync.dma_start(iota_f[:], iota_d[:, :])

            a_row = cp.tile([1, N], F32, tag="a_row")
            nc.scalar.activation(a_row[:], nu_row[:], Act.Exp)
            th2pi_row = cp.tile([1, N], F32, tag="th2pi_row")
            nc.scalar.activation(th2pi_row[:], th_row[:], Act.Exp)
            nc.scalar.activation(
                th2pi_row[:], th2pi_row[:], Act.Copy, scale=1.0 / (2.0 * PI)
            )

            # ---- C^T/-C_im^T/D^T (bf16 transposes), final projection -------
            ident = cp.tile([128, 128], BF16, tag="ident")
            identf = cp.tile([128, 128], F32, tag="identf")
            nc.sync.dma_start(identf[:], ident_d[:, :])
            nc.vector.tensor_copy(ident[:], identf[:])
            c_sb = cp.tile([O, N], F32, tag="c_sb")
            nc.sync.dma_start(c_sb[:], cre_d[:, :])
            ci_sb = cp.tile([O, N], F32, tag="ci_sb")
            nc.sync.dma_start(ci_sb[:], cim_d[:, :])
            d_sb = cp.tile([O, H], F32, tag="d_sb")
            nc.sync.dma_start(d_sb[:], dd_d[:, :])
            c_bf = cp.tile([O, N], BF16, tag="c_bf")
            nc.vector.tensor_copy(c_bf[:], c_sb[:])
            ci_bf = cp.tile([O, N], BF16, tag="ci_bf")
            nc.vector.tensor_scalar_mul(ci_bf[:], ci_sb[:], -1.0)
            d_bf = cp.tile([O, H], BF16, tag="d_bf")
            nc.vector.tensor_copy(d_bf[:], d_sb[:])

            creT = []
            ncimT = []
            for nh in range(2):
                pt = pp.tile([128, 128], BF16, tag="pv10", name=f"pt{nh}")
                nc.tensor.transpose(pt[:], c_bf[:, nh * 128 : (nh + 1) * 128], ident[:])
                t = cp.tile([128, 128], BF16, tag=f"creT{nh}", name=f"creT{nh}")
                nc.vector.tensor_copy(t[:], pt[:])
                creT.append(t)
                pt2 = pp.tile([128, 128], BF16, tag="pv11", name=f"pt2{nh}")
                nc.tensor.transpose(pt2[:], ci_bf[:, nh * 128 : (nh + 1) * 128], ident[:])
                t2 = cp.tile([128, 128], BF16, tag=f"ncimT{nh}", name=f"ncimT{nh}")
                nc.vector.tensor_copy(t2[:], pt2[:])
                ncimT.append(t2)
            ptd = pp.tile([128, 128], BF16, tag="pv10", name="ptd")
            nc.tensor.transpose(ptd[:], d_bf[:], ident[:])
            dT = cp.tile([128, 128], BF16, tag="dT")
            nc.vector.tensor_copy(dT[:], ptd[:])




            ones_row = cp.tile([1, 128], F32, tag="ones_row")
            nc.vector.memset(ones_row[:], 1.0)
            pb = pp.tile([128, N], F32, tag="pv00", name="pb")
            nc.tensor.matmul(pb[:], ones_row[:], a_row[:], start=True, stop=True)
            pb2 = pp.tile([128, N], F32, tag="pv01", name="pb2")
            nc.tensor.matmul(pb2[:], ones_row[:], th2pi_row[:], start=True, stop=True)

            # ---- u tiles: DMAs emitted early; casts split around prologue --
            uts = []
            ubs = []
            for j in range(KT):
                kk = KT - 1 - j
                u_t = up.tile([128, BS, H], F32, tag="u_t", name=f"u_t{j}")
                nc.sync.dma_start(
                    u_t[:], u_d[:, kk * 128 : (kk + 1) * 128, :].transpose([1, 0, 2])
                )
                uts.append(u_t)
                u_b = up.tile([128, BS, H], BF16, tag="u_b", name=f"u_b{j}", bufs=8)
                ubs.append(u_b)

            nc.scalar.copy(ubs[0][:], uts[0][:])
            nc.scalar.copy(ubs[1][:], uts[1][:])

            # ---- W block [128, KT, N] bf16: seed tile j=0 (kk=KT-1) --------
            wblk_re = cp.tile([128, KT, N], BF16, tag="wblk_re")
            wblk_im = cp.tile([128, KT, N], BF16, tag="wblk_im")

            c_pos_s = wk.tile([128, 1], F32, tag="c_pos")
            nc.vector.tensor_scalar(c_pos_s[:], iota_f[:], -1.0, 127.0, mult, add)
            c_neg_s = wk.tile([128, 1], F32, tag="c_neg")
            nc.vector.tensor_scalar(c_neg_s[:], iota_f[:], 1.0, -127.0, mult, add)
            mag_s = wk.tile([128, N], F32, tag="mag")
            nc.scalar.activation(mag_s[:], pb[:], Act.Exp, scale=c_neg_s[:])
            ms = wk.tile([128, N], F32, tag="ms")
            nc.scalar.activation(ms[:], pb2[:], Act.Copy, bias=0.0, scale=c_pos_s[:])
            mc = wk.tile([128, N], F32, tag="mc")
            nc.scalar.activation(mc[:], pb2[:], Act.Copy, bias=0.25, scale=c_pos_s[:])
            ims = wk.tile([128, N], I32, tag="ims")
            nc.vector.tensor_copy(ims[:], ms[:])
            imc = wk.tile([128, N], I32, tag="imc")
            nc.vector.tensor_copy(imc[:], mc[:])
            fms = wk.tile([128, N], F32, tag="fms")
            nc.vector.tensor_copy(fms[:], ims[:])
            fmc = wk.tile([128, N], F32, tag="fmc")
            nc.vector.tensor_copy(fmc[:], imc[:])
            ps = wk.tile([128, N], F32, tag="ps")
            nc.vector.tensor_tensor(ps[:], ms[:], fms[:], sub)
            pc = wk.tile([128, N], F32, tag="pc")
            nc.vector.tensor_tensor(pc[:], mc[:], fmc[:], sub)
            psm = wk.tile([128, N], F32, tag="psm")
            nc.vector.tensor_scalar(psm[:], ps[:], 0.5, None, mybir.AluOpType.is_gt)
            psw = wk.tile([128, N], F32, tag="psw")
            nc.vector.tensor_tensor(psw[:], ps[:], psm[:], sub)
            pcm = wk.tile([128, N], F32, tag="pcm")
            nc.vector.tensor_scalar(pcm[:], pc[:], 0.5, None, mybir.AluOpType.is_gt)
            pcw = wk.tile([128, N], F32, tag="pcw")
            nc.vector.tensor_tensor(pcw[:], pc[:], pcm[:], sub)
            sinv = wk.tile([128, N], F32, tag="sinv")
            nc.scalar.activation(sinv[:], psw[:], Act.Sin, scale=2.0 * PI)
            cosv = wk.tile([128, N], F32, tag="cosv")
            nc.scalar.activation(cosv[:], pcw[:], Act.Sin, scale=2.0 * PI)
            nc.vector.tensor_tensor(wblk_re[:, 0, :], mag_s[:], cosv[:], mult)
            nc.vector.tensor_tensor(wblk_im[:, 0, :], mag_s[:], sinv[:], mult)

            # ---- lam^(128*m) rows for m=1,2,4,8 (f32), bf16 broadcasts -----
            m128 = wk.tile([1, N], F32, tag="m128")
            nc.vector.tensor_scalar_mul(m128[:], th2pi_row[:], 128.0)
            m128c = wk.tile([1, N], F32, tag="m128c")
            nc.vector.tensor_scalar_add(m128c[:], m128[:], 0.25)
            i128 = wk.tile([1, N], I32, tag="i128")
            nc.vector.tensor_copy(i128[:], m128[:])
            f128 = wk.tile([1, N], F32, tag="f128")
            nc.vector.tensor_copy(f128[:], i128[:])
            r128 = wk.tile([1, N], F32, tag="r128")
            nc.vector.tensor_tensor(r128[:], m128[:], f128[:], sub)
            r128m = wk.tile([1, N], F32, tag="r128m")
            nc.vector.tensor_scalar(r128m[:], r128[:], 0.5, None, mybir.AluOpType.is_gt)
            r128w = wk.tile([1, N], F32, tag="r128w")
            nc.vector.tensor_tensor(r128w[:], r128[:], r128m[:], sub)
            i128c = wk.tile([1, N], I32, tag="i128c")
            nc.vector.tensor_copy(i128c[:], m128c[:])
            f128c = wk.tile([1, N], F32, tag="f128c")
            nc.vector.tensor_copy(f128c[:], i128c[:])
            r128c = wk.tile([1, N], F32, tag="r128c")
            nc.vector.tensor_tensor(r128c[:], m128c[:], f128c[:], sub)
            r128cm = wk.tile([1, N], F32, tag="r128cm")
            nc.vector.tensor_scalar(r128cm[:], r128c[:], 0.5, None, mybir.AluOpType.is_gt)
            r128cw = wk.tile([1, N], F32, tag="r128cw")
            nc.vector.tensor_tensor(r128cw[:], r128c[:], r128cm[:], sub)
            lsin = wk.tile([1, N], F32, tag="lsin")
            nc.scalar.activation(lsin[:], r128w[:], Act.Sin, scale=2.0 * PI)
            lcos = wk.tile([1, N], F32, tag="lcos")
            nc.scalar.activation(lcos[:], r128cw[:], Act.Sin, scale=2.0 * PI)
            mag128 = wk.tile([1, N], F32, tag="mag128")
            nc.scalar.activation(mag128[:], a_row[:], Act.Exp, scale=-128.0)

            lre = [None] * 4
            lim = [None] * 4
            lre[0] = cp.tile([1, N], F32, tag="lre0", name="lre0")
            nc.vector.tensor_tensor(lre[0][:], mag128[:], lcos[:], mult)
            lim[0] = cp.tile([1, N], F32, tag="lim0", name="lim0")
            nc.vector.tensor_tensor(lim[0][:], mag128[:], lsin[:], mult)
            for s in range(1, 4):
                # lam^(128*2^s) = (lam^(128*2^(s-1)))^2
                sq1 = wk.tile([1, N], F32, tag="sq1")
                nc.vector.tensor_tensor(sq1[:], lre[s - 1][:], lre[s - 1][:], mult)
                sq2 = wk.tile([1, N], F32, tag="sq2")
                nc.vector.tensor_tensor(sq2[:], lim[s - 1][:], lim[s - 1][:], mult)
                lre[s] = cp.tile([1, N], F32, tag=f"lre{s}", name=f"lre{s}")
                nc.vector.tensor_tensor(lre[s][:], sq1[:], sq2[:], sub)
                pr = wk.tile([1, N], F32, tag="pr")
                nc.vector.tensor_tensor(pr[:], lre[s - 1][:], lim[s - 1][:], mult)
                lim[s] = cp.tile([1, N], F32, tag=f"lim{s}", name=f"lim{s}")
                nc.vector.tensor_scalar_mul(lim[s][:], pr[:], 2.0)

            # bf16 broadcasts of lam^(128m) (bf16 matmuls also reset FP32 FWL state)
            ones_bf = cp.tile([1, 128], BF16, tag="ones_bf")
            nc.vector.memset(ones_bf[:], 1.0)
            lre_b = [None] * 4
            lim_b = [None] * 4
            for s in range(4):
                rrow = wk.tile([1, N], BF16, tag="rrow")
                nc.vector.tensor_copy(rrow[:], lre[s][:])
                pbl = pp.tile([128, N], F32, tag="pv10", name=f"pbl{s}")
                nc.tensor.matmul(pbl[:], ones_bf[:], rrow[:], start=True, stop=True)
                lre_b[s] = cp.tile([128, N], BF16, tag=f"lre_b{s}", name=f"lre_b{s}")
                nc.scalar.copy(lre_b[s][:], pbl[:])
                irow = wk.tile([1, N], BF16, tag="irow")
                nc.vector.tensor_copy(irow[:], lim[s][:])
                pbl2 = pp.tile([128, N], F32, tag="pv11", name=f"pbl2{s}")
                nc.tensor.matmul(pbl2[:], ones_bf[:], irow[:], start=True, stop=True)
                lim_b[s] = cp.tile([128, N], BF16, tag=f"lim_b{s}", name=f"lim_b{s}")
                nc.scalar.copy(lim_b[s][:], pbl2[:])

            # ---- log-doubling: W[m:2m] = W[0:m] * lam^(128m) ---------------
            for s in range(4):
                m = 1 << s
                src_re = wblk_re[:, 0:m, :]
                src_im = wblk_im[:, 0:m, :]
                Lre = lre_b[s][:, None, :].broadcast_to([128, m, N])
                Lim = lim_b[s][:, None, :].broadcast_to([128, m, N])
                q1 = wk.tile([128, m, N], BF16, tag="q1", name=f"q1_{s}", bufs=1)
                nc.vector.tensor_tensor(q1[:], src_re, Lre, mult)
                q2 = wk.tile([128, m, N], BF16, tag="q2", name=f"q2_{s}", bufs=1)
                nc.vector.tensor_tensor(q2[:], src_im, Lim, mult)
                nc.vector.tensor_tensor(wblk_re[:, m : 2 * m, :], q1[:], q2[:], sub)
                q3 = wk.tile([128, m, N], BF16, tag="q3", name=f"q3_{s}", bufs=1)
                nc.vector.tensor_tensor(q3[:], src_re, Lim, mult)
                q4 = wk.tile([128, m, N], BF16, tag="q4", name=f"q4_{s}", bufs=1)
                nc.vector.tensor_tensor(q4[:], src_im, Lre, mult)
                nc.vector.tensor_tensor(wblk_im[:, m : 2 * m, :], q3[:], q4[:], add)

            for j in range(2, KT):
                nc.scalar.copy(ubs[j][:], uts[j][:])


            # ---- B tiles (gamma-scaled), u_last^T ---------------------------
            bm_re = []
            bm_im = []
            for nh in range(2):
                g_col = cp.tile([128, 1], F32, tag=f"g_col{nh}", name=f"g_col{nh}")
                nc.sync.dma_start(g_col[:], gm_d[nh * 128 : (nh + 1) * 128][:, None])
                nc.scalar.activation(g_col[:], g_col[:], Act.Exp)
                tre = cp.tile([128, H], BF16, tag=f"bm_re{nh}", name=f"bm_re{nh}")
                tref = cp.tile([128, H], F32, tag=f"bm_ref{nh}", name=f"bm_ref{nh}")
                nc.sync.dma_start(tref[:], bre_d[nh * 128 : (nh + 1) * 128, :])
                nc.vector.tensor_scalar_mul(tre[:], tref[:], g_col[:])
                bm_re.append(tre)
                tim = cp.tile([128, H], BF16, tag=f"bm_im{nh}", name=f"bm_im{nh}")
                timf = cp.tile([128, H], F32, tag=f"bm_imf{nh}", name=f"bm_imf{nh}")
                nc.sync.dma_start(timf[:], bim_d[nh * 128 : (nh + 1) * 128, :])
                nc.vector.tensor_scalar_mul(tim[:], timf[:], g_col[:])
                bm_im.append(tim)

            ulT = cp.tile([128, BS], BF16, tag="ulT")
            ulTf = cp.tile([128, BS], F32, tag="ulTf")
            for b in range(BS):
                nc.sync.dma_start(
                    ulTf[:, b : b + 1], u_d[b, L - 1 : L, :].transpose([1, 0])
                )
            nc.vector.tensor_copy(ulT[:], ulTf[:])

            # ---- PSUM accumulators, main matmul loop ------------------------
            pv = [
                [
                    pp.tile([128, BS, H], F32, tag=f"pv{ri}{nh}", name=f"pv{ri}{nh}")
                    for nh in range(2)
                ]
                for ri in range(2)
            ]
            xre = [cp.tile([128, BS], BF16, tag=f"xre{nh}", name=f"xre{nh}") for nh in range(2)]
            xim = [cp.tile([128, BS], BF16, tag=f"xim{nh}", name=f"xim{nh}") for nh in range(2)]

            def epilogue(nh):
                bre_b = bm_re[nh][:, None, :].broadcast_to([128, BS, H])
                bim_b = bm_im[nh][:, None, :].broadcast_to([128, BS, H])
                sv0 = wk.tile([128, BS, H], BF16, tag="sv0", name=f"sv0_{nh}")
                nc.scalar.copy(sv0[:], pv[0][nh][:])
                sv1 = wk.tile([128, BS, H], BF16, tag="sv1", name=f"sv1_{nh}")
                nc.scalar.copy(sv1[:], pv[1][nh][:])
                t1 = wk.tile([128, BS, H], BF16, tag="t1", name=f"t1_{nh}")
                nc.vector.tensor_tensor(t1[:], sv0[:], bre_b, mult)
                t2 = wk.tile([128, BS, H], BF16, tag="t2", name=f"t2_{nh}")
                nc.vector.tensor_tensor(t2[:], sv1[:], bim_b, mult)
                d1 = wk.tile([128, BS, H], BF16, tag="d1", name=f"d1_{nh}")
                nc.vector.tensor_tensor(d1[:], t1[:], t2[:], sub)
                xref = wk.tile([128, BS], F32, tag="xref", name=f"xref{nh}")
                nc.vector.tensor_reduce(xref[:], d1[:], mybir.AxisListType.X, add)
                nc.vector.tensor_copy(xre[nh][:], xref[:])
                t3 = wk.tile([128, BS, H], BF16, tag="t3", name=f"t3_{nh}")
                nc.vector.tensor_tensor(t3[:], sv1[:], bre_b, mult)
                t4 = wk.tile([128, BS, H], BF16, tag="t4", name=f"t4_{nh}")
                nc.vector.tensor_tensor(t4[:], sv0[:], bim_b, mult)
                d2 = wk.tile([128, BS, H], BF16, tag="d2", name=f"d2_{nh}")
                nc.vector.tensor_tensor(d2[:], t3[:], t4[:], add)
                ximf = wk.tile([128, BS], F32, tag="ximf", name=f"ximf{nh}")
                nc.vector.tensor_reduce(ximf[:], d2[:], mybir.AxisListType.X, add)
                nc.vector.tensor_copy(xim[nh][:], ximf[:])

            def mm_group(j, nh, u_b, start, stop):
                for ri, wblk in ((0, wblk_re), (1, wblk_im)):
                    lhsT = wblk[:, j, nh * 128 : (nh + 1) * 128]
                    for half in range(2):
                        nc.tensor.matmul(
                            pv[ri][nh][:, half * 4 : (half + 1) * 4, :],
                            lhsT,
                            u_b[:, half * 4 : (half + 1) * 4, :],
                            start=start,
                            stop=stop,
                        )

            SPLIT = 8  # nh1 tiles j>=SPLIT deferred: phase B hides nh0 epilogue
            if stage >= 3:
                for j in range(KT):
                    mm_group(j, 0, ubs[j], j == 0, j == KT - 1)
                    if j < SPLIT:
                        mm_group(j, 1, ubs[j], j == 0, False)
            if stage >= 4:
                epilogue(0)
            if stage >= 3:
                for j in range(SPLIT, KT):
                    mm_group(j, 1, ubs[j], False, j == KT - 1)
            if stage >= 4:
                epilogue(1)

            if stage < 4:
                for nh in range(2):
                    nc.vector.memset(xre[nh][:], 0.001)
                    nc.vector.memset(xim[nh][:], 0.001)

            py = pp.tile([BS, O], F32, tag="pv00", name="py")
            nc.tensor.matmul(py[:], xre[0][:], creT[0][:], start=True, stop=False)
            nc.tensor.matmul(py[:], xre[1][:], creT[1][:], start=False, stop=False)
            nc.tensor.matmul(py[:], xim[0][:], ncimT[0][:], start=False, stop=False)
            nc.tensor.matmul(py[:], xim[1][:], ncimT[1][:], start=False, stop=False)
            nc.tensor.matmul(py[:], ulT[:], dT[:], start=False, stop=True)

            y_sb = cp.tile([BS, O], F32, tag="y_sb")
            nc.scalar.copy(y_sb[:], py[:])
            nc.sync.dma_start(out_d[:, :], y_sb[:])

    nc.compile()
    return nc


_NC_CACHE = None


def _get_nc():
    global _NC_CACHE
    if _NC_CACHE is None:
        _NC_CACHE = build()
    return _NC_CACHE


def _make_in_maps(inputs):
    u = np.ascontiguousarray(inputs["dynamics_disturbance_time_window"], np.float32)
    shared = {
        k: np.ascontiguousarray(inputs[k], np.float32)
        for k in (
            "nu_log", "theta_log", "gamma_log",
            "B_re", "B_im", "C_re", "C_im", "D",
        )
    }
    shared["iota128"] = np.arange(128, dtype=np.float32).reshape(128, 1)
    shared["ident128"] = np.eye(128, dtype=np.float32)
    return [
        {"u": np.ascontiguousarray(u[i * BS : (i + 1) * BS]), **shared}
        for i in range(NCORES)
    ]


def _ensure_profile_hook():
    """The agent image's antenv lacks axon_hooks; shim it and register the
    ctypes NTFF hook so run_bass_kernel_spmd(trace=True) can profile."""
    import types

    if "antenv.axon_hooks" in sys.modules:
        return
    mod = types.ModuleType("antenv.axon_hooks")
    mod._hook = None
    mod.set_axon_ntff_profile_hook = lambda h: setattr(mod, "_hook", h)
    mod.get_axon_ntff_profile_hook = lambda: mod._hook
    sys.modules["antenv.axon_hooks"] = mod
    try:
        from trn_agent_boot.trn_boot import _ntff_profile_via_ctypes

        mod._hook = _ntff_profile_via_ctypes("/opt/axon/libaxon_pjrt.so")
    except Exception as e:
        print(f"profile hook setup failed: {e}", file=sys.stderr)


def run(inputs, trace=False, tmpdir=None):
    if trace:
        _ensure_profile_hook()
    nc = _get_nc()
    in_maps = _make_in_maps(inputs)
    res = run_bass_kernel_spmd(
        nc, in_maps, list(range(NCORES)), trace=trace, tmpdir=tmpdir
    )
    out = np.concatenate([res.results[i]["out"] for i in range(NCORES)], axis=0)
    return out.astype(np.float32), res


def kernel(**inputs):
    out, _ = run(inputs, trace=False)
    return out



# revision 2
# speedup vs baseline: 2.2414x; 2.2414x over previous
"""Trainium2 Bass kernel for nn_Actor (diagonal complex LRU, last-step output).

Math: the reference scans x_t = lam*x_{t-1} + (gamma*B) u_t over L=2048 steps
and keeps y[:, -1, :].  The last state collapses to
    x_L[n] = sum_k lam[n]^k * (Bhat @ u_{L-1-k})[n]
Because |lam| <= 0.99 by construction (LRU stable init), the sum truncates:
per mode only K_n = ln(eps)/ln|lam_n| terms matter.  We sort modes by |lam|
(host-side permutation of the diagonal; the output is permutation invariant),
give the top 128 modes KT0 time-tiles of 128 steps and the bottom 128 modes
KT1 (=1) tile.  Per-core work becomes
    v[n, b, h] = sum_k W[k, n] * u[b, L-1-k, h]     (TensorE, PSUM-accum)
    d1/d2      = complex B-products of v            (VectorE, bf16)
    ypsum[o,b,h] = C-projection of d1/d2            (TensorE, contracts n)
    y[o, b]    = sum_h ypsum + D u_last             (VectorE reduce + add)
with W[k, n] = lam[n]^k precomputed on host (float64 -> bf16), u repacked on
host into contiguous time-reversed tiles (bf16), so the device does only
matmuls + a short element-wise epilogue.

Sharding: data-parallel over batch (64 -> 8 per core) on 8 NeuronCores,
no collectives; host concatenates per-core outputs.
"""

import sys

sys.path.insert(0, "/opt/trn_rl_repo")

import numpy as np

import concourse.bass as bass
import concourse.tile as tile
from concourse import bacc, mybir
from concourse.bass_utils import run_bass_kernel_spmd

try:
    from ml_dtypes import bfloat16 as np_bf16
except ImportError:  # pragma: no cover
    import jax.numpy as _jnp

    np_bf16 = _jnp.bfloat16

B, L, H, O, N = 64, 2048, 128, 128, 256
NCORES = 8
BS = B // NCORES  # 8 batches per core
EPS_TAIL = 4e-3  # per-mode truncation tail (relative); exact err ~1e-4 overall
F32 = mybir.dt.float32
BF16 = mybir.dt.bfloat16


def build(kt0=4, kt1=1):
    """Bass program for one core, parameterized by per-half tile counts."""
    nc = bacc.Bacc("TRN2", target_bir_lowering=False, debug=False)
    S = kt0 + kt1  # W slots: [0:kt0]=half0 (slow modes), [kt0:S]=half1
    KA = kt0 - 1  # tiles j>=1 for half0

    # DRAM inputs, all host-prepacked (see _plan/_make_in_maps)
    w_d = nc.dram_tensor("W", [128, 2 * S * 128], BF16, kind="ExternalInput")
    u0_d = nc.dram_tensor("u0", [128, BS * H], BF16, kind="ExternalInput")
    # uA/uB: tiles 1..kt0-1 for batch halves 0/1 (empty dim avoided if KA==0)
    ka = max(KA, 1)
    uA_d = nc.dram_tensor("uA", [128, ka * 4 * H], BF16, kind="ExternalInput")
    uB_d = nc.dram_tensor("uB", [128, ka * 4 * H], BF16, kind="ExternalInput")
    # params blob: bhat_re(2*128) bhat_im(2*128) cret(2*128) ncimt(2*128) dT(128) ulT(8)
    P2 = 8 * 128 + 128 + BS
    p_d = nc.dram_tensor("P", [128, P2], BF16, kind="ExternalInput")
    out_d = nc.dram_tensor("out", [O, BS], F32, kind="ExternalOutput")

    mult = mybir.AluOpType.mult
    add = mybir.AluOpType.add
    sub = mybir.AluOpType.subtract

    with tile.TileContext(nc) as tc:
        with (
            tc.tile_pool(name="cp", bufs=1) as cp,
            tc.tile_pool(name="psum", bufs=1, space=bass.MemorySpace.PSUM) as pp,
        ):
            # ---- DMAs: few, large, contiguous ---------------------------
            w_sb = cp.tile([128, 2, S, 128], BF16, tag="w_sb")
            nc.scalar.dma_start(w_sb[:], w_d.reshape([128, 2, S, 128])[:])
            u0_sb = cp.tile([128, BS, H], BF16, tag="u0_sb")
            nc.sync.dma_start(u0_sb[:], u0_d.reshape([128, BS, H])[:])
            uA_sb = cp.tile([128, ka, 4, H], BF16, tag="uA_sb")
            uB_sb = cp.tile([128, ka, 4, H], BF16, tag="uB_sb")
            if KA > 0:
                nc.sync.dma_start(uA_sb[:], uA_d.reshape([128, ka, 4, H])[:])
                nc.sync.dma_start(uB_sb[:], uB_d.reshape([128, ka, 4, H])[:])
            p_sb = cp.tile([128, P2], BF16, tag="p_sb")
            nc.scalar.dma_start(p_sb[:], p_d[:])

            def wre(slot):
                return w_sb[:, 0, slot, :]

            def wim(slot):
                return w_sb[:, 1, slot, :]

            bhat_re = [p_sb[:, hf * 128 : (hf + 1) * 128] for hf in range(2)]
            bhat_im = [p_sb[:, (2 + hf) * 128 : (3 + hf) * 128] for hf in range(2)]
            cret = [p_sb[:, (4 + hf) * 128 : (5 + hf) * 128] for hf in range(2)]
            ncimt = [p_sb[:, (6 + hf) * 128 : (7 + hf) * 128] for hf in range(2)]
            dT = p_sb[:, 8 * 128 : 9 * 128]
            ulT = p_sb[:, 9 * 128 : 9 * 128 + BS]

            # ---- PSUM accumulators -------------------------------------
            pv0re = pp.tile([128, BS, H], F32, tag="pv0re")
            pv0im = pp.tile([128, BS, H], F32, tag="pv0im")
            pv1re = pp.tile([128, BS, H], F32, tag="pv1re")
            pv1im = pp.tile([128, BS, H], F32, tag="pv1im")

            def urhs(j, bh):
                if j == 0:
                    return u0_sb[:, bh * 4 : (bh + 1) * 4, :]
                return (uA_sb if bh == 0 else uB_sb)[:, j - 1, :, :]

            def mains_h0(bh):
                for j in range(kt0):
                    for pv, wsl in ((pv0re, wre(j)), (pv0im, wim(j))):
                        nc.tensor.matmul(
                            pv[:, bh * 4 : (bh + 1) * 4, :],
                            wsl,
                            urhs(j, bh),
                            start=(j == 0),
                            stop=(j == kt0 - 1),
                        )

            def mains_h1(bh):
                for j in range(kt1):
                    for pv, wsl in ((pv1re, wre(kt0 + j)), (pv1im, wim(kt0 + j))):
                        nc.tensor.matmul(
                            pv[:, bh * 4 : (bh + 1) * 4, :],
                            wsl,
                            urhs(j, bh),
                            start=(j == 0),
                            stop=(j == kt1 - 1),
                        )

            # ---- epilogue helpers --------------------------------------
            # copies PSUM->SBUF bf16 on Scalar; products on Vector (2x bf16)
            def sv_copy(name, pv, bh):
                t = cp.tile([128, 4, H], BF16, tag=name, name=name)
                nc.scalar.copy(t[:], pv[:, bh * 4 : (bh + 1) * 4, :])
                return t

            def products(name, svre, svim, hf):
                bre = bhat_re[hf][:, None, :].broadcast_to([128, 4, H])
                bim = bhat_im[hf][:, None, :].broadcast_to([128, 4, H])
                q1 = cp.tile([128, 4, H], BF16, tag=f"q1{name}", name=f"q1{name}")
                nc.vector.tensor_tensor(q1[:], svre[:], bre, mult)
                q2 = cp.tile([128, 4, H], BF16, tag=f"q2{name}", name=f"q2{name}")
                nc.vector.tensor_tensor(q2[:], svim[:], bim, mult)
                d1 = cp.tile([128, 4, H], BF16, tag=f"d1{name}", name=f"d1{name}")
                nc.vector.tensor_tensor(d1[:], q1[:], q2[:], sub)
                q3 = cp.tile([128, 4, H], BF16, tag=f"q3{name}", name=f"q3{name}")
                nc.vector.tensor_tensor(q3[:], svim[:], bre, mult)
                q4 = cp.tile([128, 4, H], BF16, tag=f"q4{name}", name=f"q4{name}")
                nc.vector.tensor_tensor(q4[:], svre[:], bim, mult)
                d2 = cp.tile([128, 4, H], BF16, tag=f"d2{name}", name=f"d2{name}")
                nc.vector.tensor_tensor(d2[:], q3[:], q4[:], add)
                return d1, d2

            ypsum = pp.tile([128, BS, H], F32, tag="pv1re", name="ypsum")
            p2 = pp.tile([O, BS], F32, tag="pv1im", name="p2")

            def proj(d1, d2, hf, bh, start, stop):
                ysl = ypsum[:, bh * 4 : (bh + 1) * 4, :]
                nc.tensor.matmul(ysl, cret[hf], d1[:], start=start, stop=False)
                nc.tensor.matmul(ysl, ncimt[hf], d2[:], start=False, stop=stop)

            # ---- schedule ----------------------------------------------
            # tensor: h0-b0 mains | h1 mains | h0-b1 mains | projs | D
            mains_h0(0)
            mains_h1(0)
            mains_h1(1)

            sv_re_00 = sv_copy("sv_re_00", pv0re, 0)
            sv_im_00 = sv_copy("sv_im_00", pv0im, 0)
            d1_00, d2_00 = products("h0b0", sv_re_00, sv_im_00, 0)

            mains_h0(1)

            sv_re_10 = sv_copy("sv_re_10", pv1re, 0)
            sv_im_10 = sv_copy("sv_im_10", pv1im, 0)
            sv_re_11 = sv_copy("sv_re_11", pv1re, 1)
            sv_im_11 = sv_copy("sv_im_11", pv1im, 1)
            d1_10, d2_10 = products("h1b0", sv_re_10, sv_im_10, 1)
            d1_11, d2_11 = products("h1b1", sv_re_11, sv_im_11, 1)

            # ypsum bank b0: h0b0 (start) then h1b0 (stop)
            proj(d1_00, d2_00, 0, 0, True, False)
            proj(d1_10, d2_10, 1, 0, False, True)
            # D-term into p2 (aliases pv1im banks; waits for sv_im_1x reads)
            nc.tensor.matmul(p2[:], dT, ulT, start=True, stop=True)
            # ypsum bank b1: h1b1 (start) then h0b1 (stop)
            proj(d1_11, d2_11, 1, 1, True, False)

            sv_re_01 = sv_copy("sv_re_01", pv0re, 1)
            sv_im_01 = sv_copy("sv_im_01", pv0im, 1)
            d1_01, d2_01 = products("h0b1", sv_re_01, sv_im_01, 0)
            proj(d1_01, d2_01, 0, 1, False, True)

            # ---- final reduce over h + D add + store -------------------
            out_sb = cp.tile([O, BS], F32, tag="out_sb")
            for bh in range(2):
                ysum = cp.tile([O, 4], F32, tag=f"ysum{bh}", name=f"ysum{bh}")
                nc.vector.tensor_reduce(
                    ysum[:], ypsum[:, bh * 4 : (bh + 1) * 4, :],
                    mybir.AxisListType.X, add,
                )
                nc.vector.tensor_tensor(
                    out_sb[:, bh * 4 : (bh + 1) * 4], ysum[:],
                    p2[:, bh * 4 : (bh + 1) * 4], add,
                )
            nc.scalar.dma_start(out_d[:, :], out_sb[:])

    nc.compile()
    return nc


_NC_CACHE = {}


def _get_nc(kt0=4, kt1=1):
    key = (kt0, kt1)
    if key not in _NC_CACHE:
        _NC_CACHE[key] = build(kt0, kt1)
    return _NC_CACHE[key]


def _plan(inputs):
    """Host-side: mode sort, tile counts, lam-power tables, param packing."""
    nu = np.asarray(inputs["nu_log"], np.float64)
    th = np.asarray(inputs["theta_log"], np.float64)
    gm = np.asarray(inputs["gamma_log"], np.float64)
    lam_abs = np.exp(-np.exp(nu))
    order = np.argsort(-lam_abs)  # descending |lam|
    sl = lam_abs[order]
    K = np.ceil(np.log(EPS_TAIL) / np.log(np.minimum(sl, 1.0 - 1e-12)))
    K = np.clip(K, 1, L).astype(int)
    kt0 = max(1, int(np.ceil(K[:128].max() / 128)))
    kt1 = max(1, int(np.ceil(K[128:].max() / 128)))
    kt0 = min(kt0, L // 128)
    kt1 = min(kt1, kt0)
    S = kt0 + kt1

    lam = np.exp(-np.exp(nu[order]) + 1j * np.exp(th[order]))  # sorted modes
    # W[p, c, slot, n]: c=0 re, c=1 im; slot j covers steps k = 128j+p
    ks = np.arange(128, dtype=np.float64)
    W = np.zeros((128, 2, S, 128), np.float64)
    for j in range(kt0):
        pw = lam[:128] ** (128.0 * j + ks[:, None])  # [128p, 128n]
        W[:, 0, j, :] = pw.real
        W[:, 1, j, :] = pw.imag
    for j in range(kt1):
        pw = lam[128:] ** (128.0 * j + ks[:, None])
        W[:, 0, kt0 + j, :] = pw.real
        W[:, 1, kt0 + j, :] = pw.imag
    W_bf = W.reshape(128, 2 * S * 128).astype(np_bf16)

    Bre = np.asarray(inputs["B_re"], np.float64)[order]
    Bim = np.asarray(inputs["B_im"], np.float64)[order]
    g = np.exp(gm[order])[:, None]
    bhre, bhim = Bre * g, Bim * g  # [256, H]
    Cre = np.asarray(inputs["C_re"], np.float64)[:, order]
    Cim = np.asarray(inputs["C_im"], np.float64)[:, order]
    D = np.asarray(inputs["D"], np.float64)

    P2 = 8 * 128 + 128 + BS
    P = np.zeros((128, P2), np.float64)
    for hf in range(2):
        P[:, hf * 128 : (hf + 1) * 128] = bhre[hf * 128 : (hf + 1) * 128]
        P[:, (2 + hf) * 128 : (3 + hf) * 128] = bhim[hf * 128 : (hf + 1) * 128]
        P[:, (4 + hf) * 128 : (5 + hf) * 128] = Cre[:, hf * 128 : (hf + 1) * 128].T
        P[:, (6 + hf) * 128 : (7 + hf) * 128] = -Cim[:, hf * 128 : (hf + 1) * 128].T
    P[:, 8 * 128 : 9 * 128] = D.T
    # ulT filled per-core
    return {"kt0": kt0, "kt1": kt1, "W": W_bf, "P": P}


def _make_in_maps(inputs, plan=None):
    if plan is None:
        plan = _plan(inputs)
    kt0, kt1 = plan["kt0"], plan["kt1"]
    ka = max(kt0 - 1, 1)
    u = np.asarray(inputs["dynamics_disturbance_time_window"], np.float32)
    # time-reversed, tiled: urev[b, k, h] = u[b, L-1-k, h]; tile j rows p=k-128j
    urev = u[:, ::-1, :][:, : kt0 * 128, :].astype(np_bf16)  # [B, kt0*128, H]
    urev = urev.reshape(B, kt0, 128, H)
    in_maps = []
    for c in range(NCORES):
        ub = urev[c * BS : (c + 1) * BS]  # [BS, kt0, 128, H]
        u0 = np.ascontiguousarray(
            ub[:, 0].transpose(1, 0, 2).reshape(128, BS * H)
        )
        if kt0 > 1:
            uA = np.ascontiguousarray(
                ub[0:4, 1:].transpose(2, 1, 0, 3).reshape(128, (kt0 - 1) * 4 * H)
            )
            uB = np.ascontiguousarray(
                ub[4:8, 1:].transpose(2, 1, 0, 3).reshape(128, (kt0 - 1) * 4 * H)
            )
        else:
            uA = np.zeros((128, ka * 4 * H), np_bf16)
            uB = np.zeros((128, ka * 4 * H), np_bf16)
        P = plan["P"].copy()
        P[:, 9 * 128 : 9 * 128 + BS] = (
            u[c * BS : (c + 1) * BS, L - 1, :].astype(np.float64).T
        )
        in_maps.append(
            {
                "W": plan["W"],
                "u0": u0,
                "uA": uA,
                "uB": uB,
                "P": P.astype(np_bf16),
            }
        )
    return in_maps


def _ensure_profile_hook():
    """The agent image's antenv lacks axon_hooks; shim it and register the
    ctypes NTFF hook so run_bass_kernel_spmd(trace=True) can profile."""
    import types

    if "antenv.axon_hooks" in sys.modules:
        return
    mod = types.ModuleType("antenv.axon_hooks")
    mod._hook = None
    mod.set_axon_ntff_profile_hook = lambda h: setattr(mod, "_hook", h)
    mod.get_axon_ntff_profile_hook = lambda: mod._hook
    sys.modules["antenv.axon_hooks"] = mod
    try:
        from trn_agent_boot.trn_boot import _ntff_profile_via_ctypes

        mod._hook = _ntff_profile_via_ctypes("/opt/axon/libaxon_pjrt.so")
    except Exception as e:
        print(f"profile hook setup failed: {e}", file=sys.stderr)


def run(inputs, trace=False, tmpdir=None):
    if trace:
        _ensure_profile_hook()
    plan = _plan(inputs)
    nc = _get_nc(plan["kt0"], plan["kt1"])
    in_maps = _make_in_maps(inputs, plan)
    res = run_bass_kernel_spmd(
        nc, in_maps, list(range(NCORES)), trace=trace, tmpdir=tmpdir
    )
    out = np.concatenate(
        [np.asarray(res.results[i]["out"]).T for i in range(NCORES)], axis=0
    )
    return out.astype(np.float32), res


def kernel(**inputs):
    out, _ = run(inputs, trace=False)
    return out
